# revision 7
# baseline (speedup 1.0000x reference)
"""Trainium2 Bass kernel for the ProductManifoldGPHead problem.

mean = Kxu @ (Ku^-1 @ f_u), where Kxu is a product of a Poincare-ball RBF
(2-d) and a Euclidean RBF (64-d) over BT=8192 query rows x M=256 inducing
points.

Sharding: data-parallel over the flattened BT axis across 8 cores (1024 rows
per core). The small inducing-point quantities (u features, W = Ku^-1 @ f_u)
are computed on host and replicated, per the problem's sharding hint.

Device math per core, in transposed layout (m on partitions, n on free axis):
  K[m,n] = exp(-a*d_hyp^2 - b*d2z)
         = exp(-a*d_hyp^2 + 2b*u.x - b*|u_m|^2 - b*|x_n|^2)
  the -b*|x_n|^2 term rides as an extra contraction feature in the z matmul,
  and 1/den separates (EPS dropped - it only perturbs entries whose Kh
  underflows to 0 in fp32 anyway):
  ratio = 2*d2h/den = (d2h/(1-x2c_n)) * (2/(1-y2c_m))  <- matmul * per-part
  d = ln(arg + sqrt(max(arg^2-1, EPS))),  arg = 1 + ratio
  All ACT functions are from one table set (Ln/Exp/Square) - sqrt is
  computed as exp(0.5*ln(x)) to avoid a table switch.
  y^T[a,n] = sum_m W[m,a]*K[m,n] via one PE matmul per n-block.
"""

import numpy as np

import concourse.bacc as bacc
import concourse.tile as tile
import concourse.mybir as mybir
from concourse.bass_utils import run_bass_kernel_spmd
from concourse.mybir import (
    ActivationFunctionType as AF,
    AluOpType as ALU,
    AxisListType as AX,
)

N_CORES = 8
B, T, Z, M, A = 16, 512, 64, 256, 16
BT = B * T
NPC = BT // N_CORES          # 1024 rows per core
NBLK = 512                   # n-block width (PSUM bank)
NB = NPC // NBLK             # 2 n-blocks per core
TPB = NBLK // 128            # 4 row-tiles per n-block
T_TILES = NPC // 128         # 8 row-tiles per core
NF = Z + 5                   # transpose block: 64 xz, b|x|^2, ivn, 3 h-feats

EPS = 1e-6
BALL_EDGE = 1.0 - 1e-5
MAX_SQNORM = 1.0 - 1e-6
JITTER = 1e-5
F32 = mybir.dt.float32

_NC = None  # cached compiled program


def _build():
    nc = bacc.Bacc("TRN2", target_bir_lowering=False, debug=False,
                   num_devices=N_CORES)

    XZ = nc.dram_tensor("xz", [NPC, Z], F32, kind="ExternalInput")
    XH = nc.dram_tensor("xh", [NPC, 2], F32, kind="ExternalInput")
    UA = nc.dram_tensor("ua", [Z + 1, M], F32, kind="ExternalInput")   # [-2b u_z^T; 1]
    UC = nc.dram_tensor("uc", [5, M], F32, kind="ExternalInput")  # [0;|uh'|^2;1;-2uh']
    NBU = nc.dram_tensor("nbu", [128, 2], F32, kind="ExternalInput")  # -b|u_z|^2
    IVM = nc.dram_tensor("ivm", [128, 2], F32, kind="ExternalInput")  # 2/(1-y2c)
    LNA = nc.dram_tensor("lna", [128, 1], F32, kind="ExternalInput")  # ln(a)
    PB = nc.dram_tensor("pb", [128, 1], F32, kind="ExternalInput")    # +b
    WIN = nc.dram_tensor("w", [M, A], F32, kind="ExternalInput")      # Ku^-1 f_u
    IDN = nc.dram_tensor("idn", [128, 128], F32, kind="ExternalInput")
    Y = nc.dram_tensor("y", [A, NPC], F32, kind="ExternalOutput")     # transposed

    with tile.TileContext(nc) as tc:
        with (
            tc.tile_pool(name="const", bufs=1) as cpool,
            tc.tile_pool(name="xall", bufs=1) as xpool,
            tc.tile_pool(name="work", bufs=2) as wpool,
            tc.tile_pool(name="ep", bufs=2) as epool,
            tc.tile_pool(name="kp", bufs=1) as kpool,
            tc.tile_pool(name="ps1", bufs=2, space="PSUM") as ps1,
            tc.tile_pool(name="ps2", bufs=1, space="PSUM") as ps2,
            tc.tile_pool(name="pst", bufs=2, space="PSUM") as pst,
        ):
            ua = cpool.tile([Z + 1, M], F32)
            nc.sync.dma_start(ua[:], UA.ap())
            uc = cpool.tile([NF, M], F32)
            nc.sync.dma_start(uc[Z:NF, :], UC.ap())
            nbu = cpool.tile([128, 2], F32)
            nc.sync.dma_start(nbu[:], NBU.ap())
            ivm = cpool.tile([128, 2], F32)
            nc.sync.dma_start(ivm[:], IVM.ap())
            lna = cpool.tile([128, 1], F32)
            nc.sync.dma_start(lna[:], LNA.ap())
            pb = cpool.tile([128, 1], F32)
            nc.sync.dma_start(pb[:], PB.ap())
            w0 = cpool.tile([128, A], F32)
            nc.sync.dma_start(w0[:], WIN.ap()[0:128, :])
            w1 = cpool.tile([128, A], F32)
            nc.sync.dma_start(w1[:], WIN.ap()[128:256, :])
            idn = cpool.tile([128, 128], F32)
            nc.sync.dma_start(idn[:], IDN.ap())

            # ---- whole-core x block [128, t, NF]:
            # cols 0..63 xz | 64: b|xz|^2 | 65: ivn | 66..68: h-feats * ivn
            xz_all = xpool.tile([128, T_TILES, NF], F32)
            xzr = XZ.ap().rearrange("(t p) c -> p t c", p=128)
            half = T_TILES // 2
            nc.sync.dma_start(xz_all[:, 0:half, 0:Z], xzr[:, 0:half, :])
            nc.gpsimd.dma_start(xz_all[:, half:T_TILES, 0:Z],
                                xzr[:, half:T_TILES, :])
            xh_all = xpool.tile([128, T_TILES, 2], F32)
            nc.sync.dma_start(xh_all[:], XH.ap().rearrange("(t p) c -> p t c", p=128))

            sqz = xpool.tile([128, T_TILES, Z], F32)
            nc.vector.tensor_mul(sqz[:], xz_all[:, :, 0:Z], xz_all[:, :, 0:Z])
            z2 = xpool.tile([128, T_TILES], F32)
            nc.vector.reduce_sum(z2[:], sqz[:], axis=AX.X)
            nc.vector.tensor_scalar_mul(xz_all[:, :, Z], z2[:], pb[:])

            sqh = xpool.tile([128, T_TILES, 2], F32)
            nc.vector.tensor_mul(sqh[:], xh_all[:], xh_all[:])
            r2 = xpool.tile([128, T_TILES], F32)
            nc.vector.reduce_sum(r2[:], sqh[:], axis=AX.X)
            lr2 = xpool.tile([128, T_TILES], F32)
            nc.scalar.activation(lr2[:], r2[:], AF.Ln)
            r = xpool.tile([128, T_TILES], F32)
            nc.scalar.activation(r[:], lr2[:], AF.Exp, scale=0.5)  # sqrt(r2)
            # projection scale f: 1 if r<BE; BE/max(r,1) if r>=BE
            rmax = xpool.tile([128, T_TILES], F32)
            nc.vector.tensor_scalar_max(rmax[:], r[:], 1.0)
            iq = xpool.tile([128, T_TILES], F32)
            nc.vector.reciprocal(iq[:], rmax[:])
            gm1 = xpool.tile([128, T_TILES], F32)
            nc.vector.tensor_scalar(gm1[:], iq[:], BALL_EDGE, -1.0, ALU.mult, ALU.add)
            mask = xpool.tile([128, T_TILES], F32)
            nc.vector.tensor_scalar(mask[:], r[:], BALL_EDGE, None, ALU.is_ge)
            f_all = xpool.tile([128, T_TILES], F32)
            nc.vector.tensor_tensor(f_all[:], mask[:], gm1[:], ALU.mult)
            nc.vector.tensor_scalar_add(f_all[:], f_all[:], 1.0)
            f2 = xpool.tile([128, T_TILES], F32)
            nc.vector.tensor_mul(f2[:], f_all[:], f_all[:])
            h2p = xpool.tile([128, T_TILES], F32)    # |xh'|^2 = f^2 r2
            nc.vector.tensor_mul(h2p[:], f2[:], r2[:])
            xc = xpool.tile([128, T_TILES], F32)     # min(|xh'|^2, MAX)
            nc.vector.tensor_scalar(xc[:], h2p[:], MAX_SQNORM, None, ALU.min)
            vn = xpool.tile([128, T_TILES], F32)     # 1 - x2c
            nc.vector.tensor_scalar(vn[:], xc[:], -1.0, 1.0, ALU.mult, ALU.add)
            ivn = xpool.tile([128, T_TILES], F32)
            nc.vector.reciprocal(ivn[:], vn[:])
            nc.vector.tensor_copy(xz_all[:, :, Z + 1], ivn[:])
            nc.vector.tensor_tensor(xz_all[:, :, Z + 2], h2p[:], ivn[:], ALU.mult)
            fiv = xpool.tile([128, T_TILES], F32)
            nc.vector.tensor_tensor(fiv[:], f_all[:], ivn[:], ALU.mult)
            for t in range(T_TILES):
                nc.vector.tensor_scalar_mul(
                    xz_all[:, t, Z + 3:Z + 5], xh_all[:, t, :], fiv[:, t:t + 1])

            # ---- transpose to [NF, n] layout
            xzts = []
            for nb in range(NB):
                xzt = wpool.tile([NF, NBLK], F32, tag="xzt")
                for j in range(TPB):
                    t = nb * TPB + j
                    zps = pst.tile([NF, 128], F32, tag="pt")
                    nc.tensor.transpose(zps[:], xz_all[:, t, :], idn[:])
                    nc.vector.tensor_copy(xzt[:, j * 128:(j + 1) * 128], zps[:])
                xzts.append(xzt)

            # ---- per m-tile: matmuls + fused epilogue on [128, 1024]
            ks = []
            for m in range(2):
                msl = slice(m * 128, (m + 1) * 128)
                p1 = ps1.tile([128, NPC], F32, tag="p1")  # -2b u.x + b|x|^2
                p2 = ps2.tile([128, NPC], F32, tag="p2")  # d2h / (1-x2c)
                for nb in range(NB):
                    osl = slice(nb * NBLK, (nb + 1) * NBLK)
                    nc.tensor.matmul(p1[:, osl], ua[:, msl], xzts[nb][0:Z + 1, :],
                                     start=True, stop=True)
                    nc.tensor.matmul(p2[:, osl], uc[Z:NF, msl],
                                     xzts[nb][Z:NF, :], start=True, stop=True)

                arg = epool.tile([128, NPC], F32, tag="arg")
                nc.vector.tensor_scalar(arg[:], p2[:], ivm[:, m:m + 1], 1.0,
                                        ALU.mult, ALU.add)
                sq = epool.tile([128, NPC], F32, tag="sq")
                nc.vector.tensor_mul(sq[:], arg[:], arg[:])
                qm = epool.tile([128, NPC], F32, tag="qm")
                nc.vector.tensor_scalar(qm[:], sq[:], 1.0, EPS, ALU.subtract,
                                        ALU.max)
                lq = epool.tile([128, NPC], F32, tag="lq")
                nc.scalar.activation(lq[:], qm[:], AF.Ln)
                s_ = epool.tile([128, NPC], F32, tag="s_")
                nc.scalar.activation(s_[:], lq[:], AF.Exp, scale=0.5)  # sqrt
                w_ = epool.tile([128, NPC], F32, tag="w_")
                nc.vector.tensor_tensor(w_[:], arg[:], s_[:], ALU.add)
                d_ = epool.tile([128, NPC], F32, tag="d_")
                nc.scalar.activation(d_[:], w_[:], AF.Ln)
                ld = epool.tile([128, NPC], F32, tag="ld")
                nc.scalar.activation(ld[:], d_[:], AF.Ln)
                e1 = epool.tile([128, NPC], F32, tag="e1")
                nc.scalar.activation(e1[:], ld[:], AF.Exp, scale=2.0,
                                     bias=lna[:])
                tt2 = epool.tile([128, NPC], F32, tag="tt2")
                nc.vector.tensor_tensor(tt2[:], e1[:], p1[:], ALU.add)
                kk = kpool.tile([128, NPC], F32, tag=f"k{m}")
                nc.scalar.activation(kk[:], tt2[:], AF.Exp, scale=-1.0,
                                     bias=nbu[:, m:m + 1])
                ks.append(kk)

            # ---- output: y^T[a, n] = sum_m W[m, a] K[m, n]
            for nb in range(NB):
                osl = slice(nb * NBLK, (nb + 1) * NBLK)
                yt = pst.tile([A, NBLK], F32, tag="pt")
                nc.tensor.matmul(yt[:], w0[:], ks[0][:, osl],
                                 start=True, stop=False)
                nc.tensor.matmul(yt[:], w1[:], ks[1][:, osl],
                                 start=False, stop=True)
                ysb = epool.tile([A, NBLK], F32, tag="ysb")
                nc.vector.tensor_copy(ysb[:], yt[:])
                nc.sync.dma_start(Y.ap()[:, osl], ysb[:])

    nc.compile()
    return nc


def _project_ball_np(h):
    h = h.astype(np.float32)
    r = np.sqrt(np.sum(h * h, axis=-1, keepdims=True)).astype(np.float32)
    mask = (r >= np.float32(BALL_EDGE)).astype(np.float32)
    return (h / np.maximum(r, np.float32(1.0)) * np.float32(BALL_EDGE) * mask
            + h * (np.float32(1.0) - mask)).astype(np.float32)


def _host_w(uh, u_z, f_u, ell_h, ell_z):
    """W = Ku^-1 @ f_u in float64 (Ku built from the f32 params)."""
    uh64 = uh.astype(np.float64)
    uz64 = u_z.astype(np.float64)
    x2 = np.minimum(np.sum(uh64 * uh64, axis=-1), MAX_SQNORM)
    diff2 = np.sum((uh64[:, None, :] - uh64[None, :, :]) ** 2, axis=-1)
    den = (1.0 - x2)[:, None] * (1.0 - x2)[None, :] + EPS
    arg = 1.0 + 2.0 * diff2 / den
    d = np.log(arg + np.sqrt(np.maximum(arg * arg - 1.0, EPS)))
    Kh = np.exp(-(d * d) / (2.0 * float(ell_h) ** 2))
    dz2 = np.sum((uz64[:, None, :] - uz64[None, :, :]) ** 2, axis=-1)
    Kz = np.exp(-dz2 / (2.0 * float(ell_z) ** 2))
    Ku = Kh * Kz + JITTER * np.eye(M)
    return np.linalg.solve(Ku, f_u.astype(np.float64)).astype(np.float32)


def kernel(h_tp1, z_t, u_h, u_z, log_ell_h, log_ell_z, f_u):
    global _NC
    h_tp1 = np.asarray(h_tp1, dtype=np.float32)
    z_t = np.asarray(z_t, dtype=np.float32)
    u_h = np.asarray(u_h, dtype=np.float32)
    u_z = np.asarray(u_z, dtype=np.float32)
    f_u = np.asarray(f_u, dtype=np.float32)

    ell_h = np.float32(np.exp(np.float32(log_ell_h)))
    ell_z = np.float32(np.exp(np.float32(log_ell_z)))
    a = 1.0 / (2.0 * float(ell_h) ** 2)
    b = 1.0 / (2.0 * float(ell_z) ** 2)

    uh = _project_ball_np(u_h)
    W = _host_w(uh, u_z, f_u, ell_h, ell_z)

    uh64 = uh.astype(np.float64)
    uh2 = np.sum(uh64 * uh64, axis=-1)
    u2z = np.sum(u_z.astype(np.float64) ** 2, axis=-1)
    vm = 1.0 - np.minimum(uh2, MAX_SQNORM)                          # 1 - y2c

    UA = np.concatenate([-2.0 * b * u_z.T.astype(np.float64),
                         np.ones((1, M))]).astype(np.float32)       # [Z+1, M]
    UC = np.stack([
        np.zeros(M, np.float64),        # pairs b|x|^2 rider -> 0
        uh2,                            # pairs ivn
        np.ones(M, np.float64),         # pairs |xh'|^2 * ivn
        -2.0 * uh64[:, 0],              # pairs xh'0 * ivn
        -2.0 * uh64[:, 1],              # pairs xh'1 * ivn
    ]).astype(np.float32)                                           # [5, M]
    NBU = (-b * u2z).astype(np.float32).reshape(2, 128).T.copy()    # [128, 2]
    IVM = (2.0 / vm).astype(np.float32).reshape(2, 128).T.copy()    # [128, 2]
    LNA = np.full((128, 1), np.log(a), np.float32)
    PB = np.full((128, 1), b, np.float32)
    IDN = np.eye(128, dtype=np.float32)

    Xz = np.ascontiguousarray(z_t.reshape(BT, Z))
    Xh = np.ascontiguousarray(h_tp1.reshape(BT, 2))

    if _NC is None:
        _NC = _build()

    in_maps = []
    for c in range(N_CORES):
        rows = slice(c * NPC, (c + 1) * NPC)
        in_maps.append({
            "xz": np.ascontiguousarray(Xz[rows]),
            "xh": np.ascontiguousarray(Xh[rows]),
            "ua": UA, "uc": UC, "nbu": NBU, "ivm": IVM,
            "lna": LNA, "pb": PB, "w": W, "idn": IDN,
        })

    res = run_bass_kernel_spmd(_NC, in_maps, list(range(N_CORES)))
    yt = np.concatenate([res.results[c]["y"] for c in range(N_CORES)], axis=1)
    return np.ascontiguousarray(yt.T).reshape(B, T, A).astype(np.float32)


# revision 8
# speedup vs baseline: 1.0903x; 1.0903x over previous
"""Trainium2 Bass kernel for the ProductManifoldGPHead problem.

mean = Kxu @ (Ku^-1 @ f_u), where Kxu is a product of a Poincare-ball RBF
(2-d) and a Euclidean RBF (64-d) over BT=8192 query rows x M=256 inducing
points.

Sharding: data-parallel over the flattened BT axis across 8 cores (1024 rows
per core). The small inducing-point quantities (u features, W = Ku^-1 @ f_u)
are computed on host and replicated, per the problem's sharding hint.

Device math per core, in transposed layout (m on partitions, n on free axis):
  K[m,n] = exp(-a*d_hyp^2 - b*d2z)
         = exp(-a*d_hyp^2 + 2b*u.x - b*|u_m|^2 - b*|x_n|^2)
  the -b*|x_n|^2 term rides as an extra contraction feature in the z matmul,
  and 1/den separates (EPS dropped - it only perturbs entries whose Kh
  underflows to 0 in fp32 anyway):
  ratio = 2*d2h/den = (d2h/(1-x2c_n)) * (2/(1-y2c_m))  <- matmul * per-part
  d = ln(arg + sqrt(max(arg^2-1, EPS))),  arg = 1 + ratio
  All ACT functions are from one table set (Ln/Exp/Square) - sqrt is
  computed as exp(0.5*ln(x)) to avoid a table switch.
  y^T[a,n] = sum_m W[m,a]*K[m,n] via one PE matmul per n-block.
"""

import numpy as np

import concourse.bacc as bacc
import concourse.tile as tile
import concourse.mybir as mybir
from concourse.bass_utils import run_bass_kernel_spmd
from concourse.mybir import (
    ActivationFunctionType as AF,
    AluOpType as ALU,
    AxisListType as AX,
)

N_CORES = 8
B, T, Z, M, A = 16, 512, 64, 256, 16
BT = B * T
NPC = BT // N_CORES          # 1024 rows per core
NBLK = 512                   # n-block width (PSUM bank)
NB = NPC // NBLK             # 2 n-blocks per core
TPB = NBLK // 128            # 4 row-tiles per n-block
T_TILES = NPC // 128         # 8 row-tiles per core
NF = Z + 5                   # transpose block: 64 xz, b|x|^2, ivn, 3 h-feats

EPS = 1e-6
BALL_EDGE = 1.0 - 1e-5
MAX_SQNORM = 1.0 - 1e-6
JITTER = 1e-5
F32 = mybir.dt.float32

_NC = None  # cached compiled program


def _build():
    nc = bacc.Bacc("TRN2", target_bir_lowering=False, debug=False,
                   num_devices=N_CORES)

    XZ = nc.dram_tensor("xz", [NPC, Z], F32, kind="ExternalInput")
    XH = nc.dram_tensor("xh", [NPC, 2], F32, kind="ExternalInput")
    UA = nc.dram_tensor("ua", [Z + 1, M], F32, kind="ExternalInput")   # [-2b u_z^T; 1]
    UC = nc.dram_tensor("uc", [5, M], F32, kind="ExternalInput")  # [0;|uh'|^2;1;-2uh']
    NBU = nc.dram_tensor("nbu", [128, 2], F32, kind="ExternalInput")  # -b|u_z|^2
    IVM = nc.dram_tensor("ivm", [128, 2], F32, kind="ExternalInput")  # 2/(1-y2c)
    LNA = nc.dram_tensor("lna", [128, 1], F32, kind="ExternalInput")  # ln(a)
    PB = nc.dram_tensor("pb", [128, 1], F32, kind="ExternalInput")    # +b
    WIN = nc.dram_tensor("w", [M, A], F32, kind="ExternalInput")      # Ku^-1 f_u
    IDN = nc.dram_tensor("idn", [128, 128], F32, kind="ExternalInput")
    Y = nc.dram_tensor("y", [A, NPC], F32, kind="ExternalOutput")     # transposed

    with tile.TileContext(nc) as tc:
        with (
            tc.tile_pool(name="const", bufs=1) as cpool,
            tc.tile_pool(name="xall", bufs=1) as xpool,
            tc.tile_pool(name="work", bufs=2) as wpool,
            tc.tile_pool(name="ep", bufs=2) as epool,
            tc.tile_pool(name="kp", bufs=1) as kpool,
            tc.tile_pool(name="ps1", bufs=2, space="PSUM") as ps1,
            tc.tile_pool(name="ps2", bufs=1, space="PSUM") as ps2,
            tc.tile_pool(name="pst", bufs=2, space="PSUM") as pst,
        ):
            ua = cpool.tile([Z + 1, M], F32)
            nc.sync.dma_start(ua[:], UA.ap())
            uc = cpool.tile([NF, M], F32)
            nc.sync.dma_start(uc[Z:NF, :], UC.ap())
            nbu = cpool.tile([128, 2], F32)
            nc.sync.dma_start(nbu[:], NBU.ap())
            ivm = cpool.tile([128, 2], F32)
            nc.sync.dma_start(ivm[:], IVM.ap())
            lna = cpool.tile([128, 1], F32)
            nc.sync.dma_start(lna[:], LNA.ap())
            pb = cpool.tile([128, 1], F32)
            nc.sync.dma_start(pb[:], PB.ap())
            w0 = cpool.tile([128, A], F32)
            nc.sync.dma_start(w0[:], WIN.ap()[0:128, :])
            w1 = cpool.tile([128, A], F32)
            nc.sync.dma_start(w1[:], WIN.ap()[128:256, :])
            idn = cpool.tile([128, 128], F32)
            nc.sync.dma_start(idn[:], IDN.ap())

            # ---- whole-core x block [128, t, NF]:
            # cols 0..63 xz | 64: b|xz|^2 | 65: ivn | 66..68: h-feats * ivn
            xz_all = xpool.tile([128, T_TILES, NF], F32)
            xzr = XZ.ap().rearrange("(t p) c -> p t c", p=128)
            half = T_TILES // 2
            nc.sync.dma_start(xz_all[:, 0:half, 0:Z], xzr[:, 0:half, :])
            nc.gpsimd.dma_start(xz_all[:, half:T_TILES, 0:Z],
                                xzr[:, half:T_TILES, :])
            xh_all = xpool.tile([128, T_TILES, 2], F32)
            nc.sync.dma_start(xh_all[:], XH.ap().rearrange("(t p) c -> p t c", p=128))

            sqz = xpool.tile([128, T_TILES, Z], F32)
            nc.vector.tensor_mul(sqz[:], xz_all[:, :, 0:Z], xz_all[:, :, 0:Z])
            z2 = xpool.tile([128, T_TILES], F32)
            nc.vector.reduce_sum(z2[:], sqz[:], axis=AX.X)
            nc.vector.tensor_scalar_mul(xz_all[:, :, Z], z2[:], pb[:])

            sqh = xpool.tile([128, T_TILES, 2], F32)
            nc.vector.tensor_mul(sqh[:], xh_all[:], xh_all[:])
            r2 = xpool.tile([128, T_TILES], F32)
            nc.vector.reduce_sum(r2[:], sqh[:], axis=AX.X)
            lr2 = xpool.tile([128, T_TILES], F32)
            nc.scalar.activation(lr2[:], r2[:], AF.Ln)
            r = xpool.tile([128, T_TILES], F32)
            nc.scalar.activation(r[:], lr2[:], AF.Exp, scale=0.5)  # sqrt(r2)
            # projection scale f: 1 if r<BE; BE/max(r,1) if r>=BE
            rmax = xpool.tile([128, T_TILES], F32)
            nc.vector.tensor_scalar_max(rmax[:], r[:], 1.0)
            iq = xpool.tile([128, T_TILES], F32)
            nc.vector.reciprocal(iq[:], rmax[:])
            gm1 = xpool.tile([128, T_TILES], F32)
            nc.vector.tensor_scalar(gm1[:], iq[:], BALL_EDGE, -1.0, ALU.mult, ALU.add)
            mask = xpool.tile([128, T_TILES], F32)
            nc.vector.tensor_scalar(mask[:], r[:], BALL_EDGE, None, ALU.is_ge)
            f_all = xpool.tile([128, T_TILES], F32)
            nc.vector.tensor_tensor(f_all[:], mask[:], gm1[:], ALU.mult)
            nc.vector.tensor_scalar_add(f_all[:], f_all[:], 1.0)
            f2 = xpool.tile([128, T_TILES], F32)
            nc.vector.tensor_mul(f2[:], f_all[:], f_all[:])
            h2p = xpool.tile([128, T_TILES], F32)    # |xh'|^2 = f^2 r2
            nc.vector.tensor_mul(h2p[:], f2[:], r2[:])
            xc = xpool.tile([128, T_TILES], F32)     # min(|xh'|^2, MAX)
            nc.vector.tensor_scalar(xc[:], h2p[:], MAX_SQNORM, None, ALU.min)
            vn = xpool.tile([128, T_TILES], F32)     # 1 - x2c
            nc.vector.tensor_scalar(vn[:], xc[:], -1.0, 1.0, ALU.mult, ALU.add)
            ivn = xpool.tile([128, T_TILES], F32)
            nc.vector.reciprocal(ivn[:], vn[:])
            nc.vector.tensor_copy(xz_all[:, :, Z + 1], ivn[:])
            nc.vector.tensor_tensor(xz_all[:, :, Z + 2], h2p[:], ivn[:], ALU.mult)
            fiv = xpool.tile([128, T_TILES], F32)
            nc.vector.tensor_tensor(fiv[:], f_all[:], ivn[:], ALU.mult)
            for t in range(T_TILES):
                nc.vector.tensor_scalar_mul(
                    xz_all[:, t, Z + 3:Z + 5], xh_all[:, t, :], fiv[:, t:t + 1])

            # ---- transpose to [NF, n] layout
            xzts = []
            for nb in range(NB):
                xzt = wpool.tile([NF, NBLK], F32, tag="xzt")
                for j in range(TPB):
                    t = nb * TPB + j
                    zps = pst.tile([NF, 128], F32, tag="pt")
                    nc.tensor.transpose(zps[:], xz_all[:, t, :], idn[:])
                    nc.vector.tensor_copy(xzt[:, j * 128:(j + 1) * 128], zps[:])
                xzts.append(xzt)

            # ---- per m-tile: matmuls + fused epilogue on [128, 1024]
            ks = []
            for m in range(2):
                msl = slice(m * 128, (m + 1) * 128)
                p1 = ps1.tile([128, NPC], F32, tag="p1")  # -2b u.x + b|x|^2
                p2 = ps2.tile([128, NPC], F32, tag="p2")  # d2h / (1-x2c)
                for nb in range(NB):
                    osl = slice(nb * NBLK, (nb + 1) * NBLK)
                    nc.tensor.matmul(p1[:, osl], ua[:, msl], xzts[nb][0:Z + 1, :],
                                     start=True, stop=True)
                    nc.tensor.matmul(p2[:, osl], uc[Z:NF, msl],
                                     xzts[nb][Z:NF, :], start=True, stop=True)

                arg = epool.tile([128, NPC], F32, tag="arg")
                nc.vector.tensor_scalar(arg[:], p2[:], ivm[:, m:m + 1], 1.0,
                                        ALU.mult, ALU.add)
                sq = epool.tile([128, NPC], F32, tag="sq")
                nc.vector.tensor_mul(sq[:], arg[:], arg[:])
                qm = epool.tile([128, NPC], F32, tag="qm")
                nc.vector.tensor_scalar(qm[:], sq[:], 1.0, EPS, ALU.subtract,
                                        ALU.max)
                lq = epool.tile([128, NPC], F32, tag="lq")
                nc.scalar.activation(lq[:], qm[:], AF.Ln)
                s_ = epool.tile([128, NPC], F32, tag="s_")
                nc.scalar.activation(s_[:], lq[:], AF.Exp, scale=0.5)  # sqrt
                w_ = epool.tile([128, NPC], F32, tag="w_")
                nc.vector.tensor_tensor(w_[:], arg[:], s_[:], ALU.add)
                d_ = epool.tile([128, NPC], F32, tag="d_")
                nc.scalar.activation(d_[:], w_[:], AF.Ln)
                ld = epool.tile([128, NPC], F32, tag="ld")
                nc.scalar.activation(ld[:], d_[:], AF.Ln)
                e1 = epool.tile([128, NPC], F32, tag="e1")
                nc.scalar.activation(e1[:], ld[:], AF.Exp, scale=2.0,
                                     bias=lna[:])
                tt2 = epool.tile([128, NPC], F32, tag="tt2")
                nc.vector.tensor_tensor(tt2[:], e1[:], p1[:], ALU.add)
                kk = kpool.tile([128, NPC], F32, tag=f"k{m}")
                nc.scalar.activation(kk[:], tt2[:], AF.Exp, scale=-1.0,
                                     bias=nbu[:, m:m + 1])
                ks.append(kk)

            # ---- output: y^T[a, n] = sum_m W[m, a] K[m, n]
            for nb in range(NB):
                osl = slice(nb * NBLK, (nb + 1) * NBLK)
                yt = pst.tile([A, NBLK], F32, tag="pt")
                nc.tensor.matmul(yt[:], w0[:], ks[0][:, osl],
                                 start=True, stop=False)
                nc.tensor.matmul(yt[:], w1[:], ks[1][:, osl],
                                 start=False, stop=True)
                ysb = epool.tile([A, NBLK], F32, tag="ysb")
                nc.vector.tensor_copy(ysb[:], yt[:])
                nc.sync.dma_start(Y.ap()[:, osl], ysb[:])

    # Force every activation onto the natural_log_exp_and_others table set
    # (it contains ln/exp/square/copy/identity - everything this kernel
    # uses), so exactly one ACT_TABLE_LOAD is emitted instead of one per
    # ln<->exp alternation.
    orig_tables = bacc.get_activation_tables

    def _one_set_tables(arch):
        tabs = orig_tables(arch)
        return {name: (funcs if name == "natural_log_exp_and_others" else set())
                for name, funcs in tabs.items()}

    bacc.get_activation_tables = _one_set_tables
    try:
        nc.compile()
    finally:
        bacc.get_activation_tables = orig_tables
    return nc


def _project_ball_np(h):
    h = h.astype(np.float32)
    r = np.sqrt(np.sum(h * h, axis=-1, keepdims=True)).astype(np.float32)
    mask = (r >= np.float32(BALL_EDGE)).astype(np.float32)
    return (h / np.maximum(r, np.float32(1.0)) * np.float32(BALL_EDGE) * mask
            + h * (np.float32(1.0) - mask)).astype(np.float32)


def _host_w(uh, u_z, f_u, ell_h, ell_z):
    """W = Ku^-1 @ f_u in float64 (Ku built from the f32 params)."""
    uh64 = uh.astype(np.float64)
    uz64 = u_z.astype(np.float64)
    x2 = np.minimum(np.sum(uh64 * uh64, axis=-1), MAX_SQNORM)
    diff2 = np.sum((uh64[:, None, :] - uh64[None, :, :]) ** 2, axis=-1)
    den = (1.0 - x2)[:, None] * (1.0 - x2)[None, :] + EPS
    arg = 1.0 + 2.0 * diff2 / den
    d = np.log(arg + np.sqrt(np.maximum(arg * arg - 1.0, EPS)))
    Kh = np.exp(-(d * d) / (2.0 * float(ell_h) ** 2))
    dz2 = np.sum((uz64[:, None, :] - uz64[None, :, :]) ** 2, axis=-1)
    Kz = np.exp(-dz2 / (2.0 * float(ell_z) ** 2))
    Ku = Kh * Kz + JITTER * np.eye(M)
    return np.linalg.solve(Ku, f_u.astype(np.float64)).astype(np.float32)


def kernel(h_tp1, z_t, u_h, u_z, log_ell_h, log_ell_z, f_u):
    global _NC
    h_tp1 = np.asarray(h_tp1, dtype=np.float32)
    z_t = np.asarray(z_t, dtype=np.float32)
    u_h = np.asarray(u_h, dtype=np.float32)
    u_z = np.asarray(u_z, dtype=np.float32)
    f_u = np.asarray(f_u, dtype=np.float32)

    ell_h = np.float32(np.exp(np.float32(log_ell_h)))
    ell_z = np.float32(np.exp(np.float32(log_ell_z)))
    a = 1.0 / (2.0 * float(ell_h) ** 2)
    b = 1.0 / (2.0 * float(ell_z) ** 2)

    uh = _project_ball_np(u_h)
    W = _host_w(uh, u_z, f_u, ell_h, ell_z)

    uh64 = uh.astype(np.float64)
    uh2 = np.sum(uh64 * uh64, axis=-1)
    u2z = np.sum(u_z.astype(np.float64) ** 2, axis=-1)
    vm = 1.0 - np.minimum(uh2, MAX_SQNORM)                          # 1 - y2c

    UA = np.concatenate([-2.0 * b * u_z.T.astype(np.float64),
                         np.ones((1, M))]).astype(np.float32)       # [Z+1, M]
    UC = np.stack([
        np.zeros(M, np.float64),        # pairs b|x|^2 rider -> 0
        uh2,                            # pairs ivn
        np.ones(M, np.float64),         # pairs |xh'|^2 * ivn
        -2.0 * uh64[:, 0],              # pairs xh'0 * ivn
        -2.0 * uh64[:, 1],              # pairs xh'1 * ivn
    ]).astype(np.float32)                                           # [5, M]
    NBU = (-b * u2z).astype(np.float32).reshape(2, 128).T.copy()    # [128, 2]
    IVM = (2.0 / vm).astype(np.float32).reshape(2, 128).T.copy()    # [128, 2]
    LNA = np.full((128, 1), np.log(a), np.float32)
    PB = np.full((128, 1), b, np.float32)
    IDN = np.eye(128, dtype=np.float32)

    Xz = np.ascontiguousarray(z_t.reshape(BT, Z))
    Xh = np.ascontiguousarray(h_tp1.reshape(BT, 2))

    if _NC is None:
        _NC = _build()

    in_maps = []
    for c in range(N_CORES):
        rows = slice(c * NPC, (c + 1) * NPC)
        in_maps.append({
            "xz": np.ascontiguousarray(Xz[rows]),
            "xh": np.ascontiguousarray(Xh[rows]),
            "ua": UA, "uc": UC, "nbu": NBU, "ivm": IVM,
            "lna": LNA, "pb": PB, "w": W, "idn": IDN,
        })

    res = run_bass_kernel_spmd(_NC, in_maps, list(range(N_CORES)))
    yt = np.concatenate([res.results[c]["y"] for c in range(N_CORES)], axis=1)
    return np.ascontiguousarray(yt.T).reshape(B, T, A).astype(np.float32)


# revision 10
# speedup vs baseline: 1.1764x; 1.0789x over previous
"""Trainium2 Bass kernel for the ProductManifoldGPHead problem.

mean = Kxu @ (Ku^-1 @ f_u), where Kxu is a product of a Poincare-ball RBF
(2-d) and a Euclidean RBF (64-d) over BT=8192 query rows x M=256 inducing
points.

Sharding: data-parallel over the flattened BT axis across 8 cores (1024 rows
per core). The small inducing-point quantities (u features, W = Ku^-1 @ f_u)
are computed on host and replicated, per the problem's sharding hint.

Device math per core, in transposed layout (m on partitions, n on free axis):
  K[m,n] = exp(-a*d_hyp^2 - b*d2z)
         = exp(-a*d_hyp^2 + 2b*u.x - b*|u_m|^2 - b*|x_n|^2)
  the -b*|x_n|^2 term rides as an extra contraction feature in the z matmul,
  and 1/den separates (EPS dropped - it only perturbs entries whose Kh
  underflows to 0 in fp32 anyway):
  ratio = 2*d2h/den = (d2h/(1-x2c_n)) * (2/(1-y2c_m))  <- matmul * per-part
  d = ln(arg + sqrt(max(arg^2-1, EPS))),  arg = 1 + ratio
  All ACT functions are from one table set (Ln/Exp/Square) - sqrt is
  computed as exp(0.5*ln(x)) to avoid a table switch.
  y^T[a,n] = sum_m W[m,a]*K[m,n] via one PE matmul per n-block.
"""

import numpy as np

import concourse.bacc as bacc
import concourse.tile as tile
import concourse.mybir as mybir
from concourse.bass_utils import run_bass_kernel_spmd
from concourse.mybir import (
    ActivationFunctionType as AF,
    AluOpType as ALU,
    AxisListType as AX,
)

N_CORES = 8
B, T, Z, M, A = 16, 512, 64, 256, 16
BT = B * T
NPC = BT // N_CORES          # 1024 rows per core
NBLK = 512                   # n-block width (PSUM bank)
NB = NPC // NBLK             # 2 n-blocks per core
TPB = NBLK // 128            # 4 row-tiles per n-block
T_TILES = NPC // 128         # 8 row-tiles per core
NF = Z + 5                   # transpose block: 64 xz, b|x|^2, ivn, 3 h-feats

EPS = 1e-6
BALL_EDGE = 1.0 - 1e-5
MAX_SQNORM = 1.0 - 1e-6
JITTER = 1e-5
F32 = mybir.dt.float32

_NC = None  # cached compiled program


def _build():
    nc = bacc.Bacc("TRN2", target_bir_lowering=False, debug=False,
                   num_devices=N_CORES)

    XZ = nc.dram_tensor("xz", [NPC, Z], F32, kind="ExternalInput")
    XH = nc.dram_tensor("xh", [NPC, 2], F32, kind="ExternalInput")
    UA = nc.dram_tensor("ua", [Z + 1, M], F32, kind="ExternalInput")   # [-2b u_z^T; 1]
    UC = nc.dram_tensor("uc", [5, M], F32, kind="ExternalInput")  # [0;|uh'|^2;1;-2uh']
    NBU = nc.dram_tensor("nbu", [128, 2], F32, kind="ExternalInput")  # -b|u_z|^2
    IVM = nc.dram_tensor("ivm", [128, 2], F32, kind="ExternalInput")  # 2/(1-y2c)
    SQA = nc.dram_tensor("sqa", [128, 1], F32, kind="ExternalInput")  # sqrt(a)
    PB = nc.dram_tensor("pb", [128, 1], F32, kind="ExternalInput")    # +b
    WIN = nc.dram_tensor("w", [M, A], F32, kind="ExternalInput")      # Ku^-1 f_u
    IDN = nc.dram_tensor("idn", [128, 128], F32, kind="ExternalInput")
    Y = nc.dram_tensor("y", [A, NPC], F32, kind="ExternalOutput")     # transposed

    with tile.TileContext(nc) as tc:
        with (
            tc.tile_pool(name="const", bufs=1) as cpool,
            tc.tile_pool(name="xall", bufs=1) as xpool,
            tc.tile_pool(name="work", bufs=2) as wpool,
            tc.tile_pool(name="ep", bufs=2) as epool,
            tc.tile_pool(name="kp", bufs=1) as kpool,
            tc.tile_pool(name="ps1", bufs=4, space="PSUM") as ps1,
            tc.tile_pool(name="ps2", bufs=2, space="PSUM") as ps2,
            tc.tile_pool(name="pst", bufs=2, space="PSUM") as pst,
        ):
            ua = cpool.tile([Z + 1, M], F32)
            nc.sync.dma_start(ua[:], UA.ap())
            uc = cpool.tile([NF, M], F32)
            nc.sync.dma_start(uc[Z:NF, :], UC.ap())
            nbu = cpool.tile([128, 2], F32)
            nc.sync.dma_start(nbu[:], NBU.ap())
            ivm = cpool.tile([128, 2], F32)
            nc.sync.dma_start(ivm[:], IVM.ap())
            sqa = cpool.tile([128, 1], F32)
            nc.sync.dma_start(sqa[:], SQA.ap())
            pb = cpool.tile([128, 1], F32)
            nc.sync.dma_start(pb[:], PB.ap())
            w0 = cpool.tile([128, A], F32)
            nc.sync.dma_start(w0[:], WIN.ap()[0:128, :])
            w1 = cpool.tile([128, A], F32)
            nc.sync.dma_start(w1[:], WIN.ap()[128:256, :])
            idn = cpool.tile([128, 128], F32)
            nc.sync.dma_start(idn[:], IDN.ap())
            epsb = cpool.tile([128, 1], F32)
            nc.gpsimd.memset(epsb[:], EPS)

            # ---- per-n-block x data + batched per-row stats
            xzs, xhs = [], []
            for nb in range(NB):
                xz_nb = xpool.tile([128, TPB, NF], F32, tag=f"xz{nb}")
                xh_nb = xpool.tile([128, TPB, 2], F32, tag=f"xh{nb}")
                xzr = XZ.ap().rearrange("(t p) c -> p t c", p=128)
                xhr = XH.ap().rearrange("(t p) c -> p t c", p=128)
                tsl = slice(nb * TPB, (nb + 1) * TPB)
                eng = nc.sync if nb == 0 else nc.gpsimd
                eng.dma_start(xz_nb[:, :, 0:Z], xzr[:, tsl, :])
                eng.dma_start(xh_nb[:], xhr[:, tsl, :])
                xzs.append(xz_nb)
                xhs.append(xh_nb)

            z2 = xpool.tile([128, T_TILES], F32)
            r2 = xpool.tile([128, T_TILES], F32)
            for nb in range(NB):
                csl = slice(nb * TPB, (nb + 1) * TPB)
                sqz = xpool.tile([128, TPB, Z], F32, tag="sqz")
                nc.vector.tensor_mul(sqz[:], xzs[nb][:, :, 0:Z], xzs[nb][:, :, 0:Z])
                nc.vector.reduce_sum(z2[:, csl], sqz[:], axis=AX.X)
                sqh = xpool.tile([128, TPB, 2], F32, tag="sqh")
                nc.vector.tensor_mul(sqh[:], xhs[nb][:], xhs[nb][:])
                nc.vector.reduce_sum(r2[:, csl], sqh[:], axis=AX.X)

            lr2 = xpool.tile([128, T_TILES], F32)
            nc.scalar.activation(lr2[:], r2[:], AF.Ln)
            r = xpool.tile([128, T_TILES], F32)
            nc.scalar.activation(r[:], lr2[:], AF.Exp, scale=0.5)  # sqrt(r2)
            # projection scale f: 1 if r<BE; BE/max(r,1) if r>=BE
            rmax = xpool.tile([128, T_TILES], F32)
            nc.vector.tensor_scalar_max(rmax[:], r[:], 1.0)
            iq = xpool.tile([128, T_TILES], F32)
            nc.vector.reciprocal(iq[:], rmax[:])
            gm1 = xpool.tile([128, T_TILES], F32)
            nc.vector.tensor_scalar(gm1[:], iq[:], BALL_EDGE, -1.0, ALU.mult, ALU.add)
            mask = xpool.tile([128, T_TILES], F32)
            nc.vector.tensor_scalar(mask[:], r[:], BALL_EDGE, None, ALU.is_ge)
            f_all = xpool.tile([128, T_TILES], F32)
            nc.vector.tensor_tensor(f_all[:], mask[:], gm1[:], ALU.mult)
            nc.vector.tensor_scalar_add(f_all[:], f_all[:], 1.0)
            f2 = xpool.tile([128, T_TILES], F32)
            nc.vector.tensor_mul(f2[:], f_all[:], f_all[:])
            h2p = xpool.tile([128, T_TILES], F32)    # |xh'|^2 = f^2 r2
            nc.vector.tensor_mul(h2p[:], f2[:], r2[:])
            xc = xpool.tile([128, T_TILES], F32)     # min(|xh'|^2, MAX)
            nc.vector.tensor_scalar(xc[:], h2p[:], MAX_SQNORM, None, ALU.min)
            vn = xpool.tile([128, T_TILES], F32)     # 1 - x2c
            nc.vector.tensor_scalar(vn[:], xc[:], -1.0, 1.0, ALU.mult, ALU.add)
            ivn = xpool.tile([128, T_TILES], F32)
            nc.vector.reciprocal(ivn[:], vn[:])
            fiv = xpool.tile([128, T_TILES], F32)
            nc.vector.tensor_tensor(fiv[:], f_all[:], ivn[:], ALU.mult)

            # rider columns + transposes + matmuls + epilogue, per (nb, m)
            xzts = []
            for nb in range(NB):
                csl = slice(nb * TPB, (nb + 1) * TPB)
                nc.vector.tensor_scalar_mul(xzs[nb][:, :, Z], z2[:, csl], pb[:])
                nc.vector.tensor_copy(xzs[nb][:, :, Z + 1], ivn[:, csl])
                nc.vector.tensor_tensor(xzs[nb][:, :, Z + 2], h2p[:, csl],
                                        ivn[:, csl], ALU.mult)
                for j in range(TPB):
                    t = nb * TPB + j
                    nc.vector.tensor_scalar_mul(
                        xzs[nb][:, j, Z + 3:Z + 5], xhs[nb][:, j, :],
                        fiv[:, t:t + 1])
                xzt = wpool.tile([NF, NBLK], F32, tag=f"xzt{nb}")
                for j in range(TPB):
                    zps = pst.tile([NF, 128], F32, tag="pt")
                    nc.tensor.transpose(zps[:], xzs[nb][:, j, :], idn[:])
                    if j % 2 == 0:
                        nc.vector.tensor_copy(xzt[:, j * 128:(j + 1) * 128], zps[:])
                    else:
                        nc.scalar.copy(xzt[:, j * 128:(j + 1) * 128], zps[:])
                xzts.append(xzt)

            ks = {}
            for nb in range(NB):
                for m in range(2):
                    msl = slice(m * 128, (m + 1) * 128)
                    p1 = ps1.tile([128, NBLK], F32, tag="p1")
                    p2 = ps2.tile([128, NBLK], F32, tag="p2")
                    nc.tensor.matmul(p1[:], ua[:, msl], xzts[nb][0:Z + 1, :],
                                     start=True, stop=True)
                    nc.tensor.matmul(p2[:], uc[Z:NF, msl], xzts[nb][Z:NF, :],
                                     start=True, stop=True)

                    ratio = epool.tile([128, NBLK], F32, tag="ratio")
                    nc.vector.tensor_scalar_mul(ratio[:], p2[:], ivm[:, m:m + 1])
                    q = epool.tile([128, NBLK], F32, tag="q")
                    nc.vector.scalar_tensor_tensor(q[:], ratio[:], 2.0, ratio[:],
                                                   ALU.add, ALU.mult)
                    lq = epool.tile([128, NBLK], F32, tag="lq")
                    nc.scalar.activation(lq[:], q[:], AF.Ln, bias=epsb[:])
                    s_ = epool.tile([128, NBLK], F32, tag="s_")
                    nc.scalar.activation(s_[:], lq[:], AF.Exp, scale=0.5)
                    w_ = epool.tile([128, NBLK], F32, tag="w_")
                    nc.vector.scalar_tensor_tensor(w_[:], ratio[:], 1.0, s_[:],
                                                   ALU.add, ALU.add)
                    d_ = epool.tile([128, NBLK], F32, tag="d_")
                    nc.scalar.activation(d_[:], w_[:], AF.Ln)
                    e1 = epool.tile([128, NBLK], F32, tag="e1")
                    nc.scalar.activation(e1[:], d_[:], AF.Square, scale=sqa[:])
                    tt2 = epool.tile([128, NBLK], F32, tag="tt2")
                    nc.vector.tensor_tensor(tt2[:], e1[:], p1[:], ALU.add)
                    kk = kpool.tile([128, NBLK], F32, tag=f"k{m}{nb}")
                    nc.scalar.activation(kk[:], tt2[:], AF.Exp, scale=-1.0,
                                         bias=nbu[:, m:m + 1])
                    ks[(m, nb)] = kk

            # ---- output: y^T[a, n] = sum_m W[m, a] K[m, n]
            for nb in range(NB):
                osl = slice(nb * NBLK, (nb + 1) * NBLK)
                yt = pst.tile([A, NBLK], F32, tag="pt")
                nc.tensor.matmul(yt[:], w0[:], ks[(0, nb)][:],
                                 start=True, stop=False)
                nc.tensor.matmul(yt[:], w1[:], ks[(1, nb)][:],
                                 start=False, stop=True)
                ysb = epool.tile([A, NBLK], F32, tag="ysb")
                nc.vector.tensor_copy(ysb[:], yt[:])
                nc.sync.dma_start(Y.ap()[:, osl], ysb[:])

    # Force every activation onto the natural_log_exp_and_others table set
    # (it contains ln/exp/square/copy/identity - everything this kernel
    # uses), so exactly one ACT_TABLE_LOAD is emitted instead of one per
    # ln<->exp alternation.
    orig_tables = bacc.get_activation_tables

    def _one_set_tables(arch):
        tabs = orig_tables(arch)
        return {name: (funcs if name == "natural_log_exp_and_others" else set())
                for name, funcs in tabs.items()}

    bacc.get_activation_tables = _one_set_tables
    try:
        nc.compile()
    finally:
        bacc.get_activation_tables = orig_tables
    return nc


def _project_ball_np(h):
    h = h.astype(np.float32)
    r = np.sqrt(np.sum(h * h, axis=-1, keepdims=True)).astype(np.float32)
    mask = (r >= np.float32(BALL_EDGE)).astype(np.float32)
    return (h / np.maximum(r, np.float32(1.0)) * np.float32(BALL_EDGE) * mask
            + h * (np.float32(1.0) - mask)).astype(np.float32)


def _host_w(uh, u_z, f_u, ell_h, ell_z):
    """W = Ku^-1 @ f_u in float64 (Ku built from the f32 params)."""
    uh64 = uh.astype(np.float64)
    uz64 = u_z.astype(np.float64)
    x2 = np.minimum(np.sum(uh64 * uh64, axis=-1), MAX_SQNORM)
    diff2 = np.sum((uh64[:, None, :] - uh64[None, :, :]) ** 2, axis=-1)
    den = (1.0 - x2)[:, None] * (1.0 - x2)[None, :] + EPS
    arg = 1.0 + 2.0 * diff2 / den
    d = np.log(arg + np.sqrt(np.maximum(arg * arg - 1.0, EPS)))
    Kh = np.exp(-(d * d) / (2.0 * float(ell_h) ** 2))
    dz2 = np.sum((uz64[:, None, :] - uz64[None, :, :]) ** 2, axis=-1)
    Kz = np.exp(-dz2 / (2.0 * float(ell_z) ** 2))
    Ku = Kh * Kz + JITTER * np.eye(M)
    return np.linalg.solve(Ku, f_u.astype(np.float64)).astype(np.float32)


def kernel(h_tp1, z_t, u_h, u_z, log_ell_h, log_ell_z, f_u):
    global _NC
    h_tp1 = np.asarray(h_tp1, dtype=np.float32)
    z_t = np.asarray(z_t, dtype=np.float32)
    u_h = np.asarray(u_h, dtype=np.float32)
    u_z = np.asarray(u_z, dtype=np.float32)
    f_u = np.asarray(f_u, dtype=np.float32)

    ell_h = np.float32(np.exp(np.float32(log_ell_h)))
    ell_z = np.float32(np.exp(np.float32(log_ell_z)))
    a = 1.0 / (2.0 * float(ell_h) ** 2)
    b = 1.0 / (2.0 * float(ell_z) ** 2)

    uh = _project_ball_np(u_h)
    W = _host_w(uh, u_z, f_u, ell_h, ell_z)

    uh64 = uh.astype(np.float64)
    uh2 = np.sum(uh64 * uh64, axis=-1)
    u2z = np.sum(u_z.astype(np.float64) ** 2, axis=-1)
    vm = 1.0 - np.minimum(uh2, MAX_SQNORM)                          # 1 - y2c

    UA = np.concatenate([-2.0 * b * u_z.T.astype(np.float64),
                         np.ones((1, M))]).astype(np.float32)       # [Z+1, M]
    UC = np.stack([
        np.zeros(M, np.float64),        # pairs b|x|^2 rider -> 0
        uh2,                            # pairs ivn
        np.ones(M, np.float64),         # pairs |xh'|^2 * ivn
        -2.0 * uh64[:, 0],              # pairs xh'0 * ivn
        -2.0 * uh64[:, 1],              # pairs xh'1 * ivn
    ]).astype(np.float32)                                           # [5, M]
    NBU = (-b * u2z).astype(np.float32).reshape(2, 128).T.copy()    # [128, 2]
    IVM = (2.0 / vm).astype(np.float32).reshape(2, 128).T.copy()    # [128, 2]
    SQA = np.full((128, 1), np.sqrt(a), np.float32)
    PB = np.full((128, 1), b, np.float32)
    IDN = np.eye(128, dtype=np.float32)

    Xz = np.ascontiguousarray(z_t.reshape(BT, Z))
    Xh = np.ascontiguousarray(h_tp1.reshape(BT, 2))

    if _NC is None:
        _NC = _build()

    in_maps = []
    for c in range(N_CORES):
        rows = slice(c * NPC, (c + 1) * NPC)
        in_maps.append({
            "xz": np.ascontiguousarray(Xz[rows]),
            "xh": np.ascontiguousarray(Xh[rows]),
            "ua": UA, "uc": UC, "nbu": NBU, "ivm": IVM,
            "sqa": SQA, "pb": PB, "w": W, "idn": IDN,
        })

    res = run_bass_kernel_spmd(_NC, in_maps, list(range(N_CORES)))
    yt = np.concatenate([res.results[c]["y"] for c in range(N_CORES)], axis=1)
    return np.ascontiguousarray(yt.T).reshape(B, T, A).astype(np.float32)
